# revision 22
# baseline (speedup 1.0000x reference)
"""Trainium2 Bass kernel for nn_EDDeform (deformable-conv CNN).

Sharding: 8 cores = (batch b in 0..3) x (output-row half h in 0..1).
Each core computes output rows [64h, 64h+64) of sample b from a padded
x slab, running the full offset chain (conv0 -> maxpool -> conv1 ->
conv2 -> conv3) and the deformable conv.

Deformable gather: offsets satisfy |off| < 1 for these inputs, so the
bilinear sample is a 3x3 "hat" stencil with separable weights
  wy[-1] = relu(-oy), wy[+1] = relu(oy), wy[0] = 1 - |oy|
(all statically addressed), folded into 9 PSUM-accumulated matmuls of
K=576 (5 chunks of 128 = k-major (k,c) pairs) against per-k shifted
copies of x (X9 windows). wy[0] is computed negated (one fused DVE op);
the sign is absorbed by negated deform weights for the 4 affected
corners.

Schedule: dma_start blocks the issuing engine's queue, so queues have
strict roles -- Scalar runs only compute (psum drains, offsets, hats),
Sync carries the small slab/weight DMAs, GpSimd streams the 9 shifted
X9 copies (deform-only).  conv0 runs from two double-shift-replicated
slab tiles (K=128 pairs, no X9 dependency), conv2 reads c1out windows
directly (no im2col gather), and conv3-t0's offsets + the first hat
block are emitted mid-conv2 so DVE deform work starts early.  Hats for
unit u+1 are produced during unit u's z muls; slab PSUM drains overlap
the next slab's first hats.
"""
import sys
import numpy as np

if "/opt/trn_rl_repo" not in sys.path:
    sys.path.insert(0, "/opt/trn_rl_repo")

import ml_dtypes
import concourse.bass as bass
import concourse.bacc as bacc
import concourse.tile as tile
import concourse.mybir as mybir
from concourse.bass_utils import run_bass_kernel_spmd

BF16 = ml_dtypes.bfloat16
F32 = np.float32
DT_BF = mybir.dt.bfloat16
DT_F32 = mybir.dt.float32
ALU = mybir.AluOpType
ACTF = mybir.ActivationFunctionType

KK = 9
NCORES = 8

_CACHE = {}


# ----------------------------------------------------------------------------
# Host-side preprocessing (sharding + weight layout), numpy only.
# ----------------------------------------------------------------------------

def _ck(idx):
    return idx // 64, idx % 64  # k, c (k-major)


def host_prepro(inputs):
    x = np.asarray(inputs["x"], F32)          # [4, 64, 130, 130]
    B, C, H, W = x.shape

    xslabs = []
    for core in range(NCORES):
        b, h = core // 2, core % 2
        slab = np.zeros((C, 70, 132), F32)
        r0 = 64 * h - 2
        lo = max(0, -r0)
        hi = min(70, H - r0)
        slab[:, lo:hi, 1:131] = x[b, :, r0 + lo:r0 + hi, :]
        # deinterleave columns: [C, 2, 70, 66], plane hh holds cols 2*jc+hh
        slab = slab.reshape(C, 70, 66, 2).transpose(0, 3, 1, 2).reshape(C, 9240)
        slabp = np.zeros((C, 9376), F32)
        slabp[:, :9240] = slab
        xslabs.append(slabp.astype(BF16))

    w0, b0 = np.asarray(inputs["w0"], F32), np.asarray(inputs["b0"], F32)
    wd = np.asarray(inputs["wd"], F32)
    wdt9 = np.zeros((128, 5 * 64), F32)
    for t in range(5):
        for p in range(128):
            idx = 128 * t + p
            if idx < 576:
                k, c = _ck(idx)
                wdt9[p, 64 * t:64 * t + 64] = wd[:, c, k // 3, k % 3]

    w1, b1 = np.asarray(inputs["w1"], F32), np.asarray(inputs["b1"], F32)
    w1t = np.zeros((65, 32), F32)
    w1t[:64] = w1[:, :, 0, 0].T
    w1t[64] = b1

    w2, b2 = np.asarray(inputs["w2"], F32), np.asarray(inputs["b2"], F32)
    w3, b3 = np.asarray(inputs["w3"], F32), np.asarray(inputs["b3"], F32)
    w3ty = np.zeros((32, 5 * 128), F32)
    w3tx = np.zeros((32, 5 * 128), F32)
    b3y = np.zeros((128, 5), F32)
    b3x = np.zeros((128, 5), F32)
    for t in range(5):
        for p in range(128):
            idx = 128 * t + p
            if idx < 576:
                k, c = _ck(idx)
                chy = (c * KK + k) * 2
                w3ty[:, 128 * t + p] = w3[chy, :, 0, 0]
                w3tx[:, 128 * t + p] = w3[chy + 1, :, 0, 0]
                b3y[p, t] = b3[chy]
                b3x[p, t] = b3[chy + 1]

    masks = []
    for core in range(NCORES):
        h = core % 2
        m = np.ones((32, 34), F32)
        m[:, 0 if h == 0 else 33] = 0.0
        masks.append(m)

    fill = np.zeros((64, 11232), F32)
    fill[:, :8976] = 1.0

    # packed bf16 weights: w0pairs|w0k8|wdp|wdn|w1t|w3ty|w3tx|w2k
    wpack = np.zeros((128, 2656), F32)
    for i in range(4):  # conv0 K=128 pair lhsT: rows (c | c+64) = k=2i | 2i+1
        k0, k1 = 2 * i, 2 * i + 1
        wpack[0:64, 64 * i:64 * i + 64] = w0[:, :, k0 // 3, k0 % 3].T
        wpack[64:128, 64 * i:64 * i + 64] = w0[:, :, k1 // 3, k1 % 3].T
    wpack[0:64, 256:320] = w0[:, :, 2, 2].T  # k8
    wpack[:, 320:640] = wdt9
    wpack[:, 640:960] = -wdt9
    wpack[0:65, 1056:1088] = w1t
    wpack[0:32, 1088:1728] = w3ty
    wpack[0:32, 1728:2368] = w3tx
    for k in range(KK):  # conv2 per-k lhsT [c2, cout2]
        wpack[0:32, 2368 + 32 * k:2368 + 32 * k + 32] = w2[:, :, k // 3, k % 3].T

    bc = np.zeros((64, 2), F32)
    bc[:, 0] = b0
    bc[0:32, 1] = b2

    const = dict(
        wpack=wpack.astype(BF16), b3y=b3y, b3x=b3x, bc=bc,
        fill=fill.astype(BF16),
    )
    in_maps = []
    for core in range(NCORES):
        m = dict(const)
        m["xslab"] = xslabs[core]
        m["maskrow"] = masks[core]
        in_maps.append(m)
    return in_maps


# ----------------------------------------------------------------------------
# Bass kernel builder.
# ----------------------------------------------------------------------------

# corners grouped by ry so cw(ry=-1) (ready after 5 hat acts) goes first
CORNERS = [(-1, -1), (-1, 1), (1, -1), (1, 1), (-1, 0),
           (1, 0), (0, -1), (0, 1), (0, 0)]


def build_nc():
    nc = bacc.Bacc(None)

    xslab_d = nc.declare_dram_parameter("xslab", [64, 9376], DT_BF, isOutput=False)
    wpack_d = nc.declare_dram_parameter("wpack", [128, 2656], DT_BF, isOutput=False)
    bc_d = nc.declare_dram_parameter("bc", [64, 2], DT_F32, isOutput=False)
    b3y_d = nc.declare_dram_parameter("b3y", [128, 5], DT_F32, isOutput=False)
    b3x_d = nc.declare_dram_parameter("b3x", [128, 5], DT_F32, isOutput=False)
    mask_d = nc.declare_dram_parameter("maskrow", [32, 34], DT_F32, isOutput=False)
    fill_d = nc.declare_dram_parameter("fill", [64, 11232], DT_BF, isOutput=False)
    out_d = nc.declare_dram_parameter("out", [64, 64 * 128], DT_BF, isOutput=True)

    with tile.TileContext(nc) as tc:
        _body(nc, tc, xslab_d, wpack_d, bc_d, b3y_d, b3x_d, mask_d, fill_d,
              out_d)
    nc.compile()
    return nc


def _body(nc, tc, xslab_d, wpack_d, bc_d, b3y_d, b3x_d, mask_d, fill_d, out_d):
    from contextlib import ExitStack

    with ExitStack() as top:
        pw = top.enter_context(tc.tile_pool(name="weights", bufs=1))
        pp = top.enter_context(tc.tile_pool(name="persist", bufs=1))

        # ---- weights (one packed DMA) ----
        wpk = pw.tile([128, 2656], DT_BF, tag="wpack")
        w0p = wpk[:][:, 0:256]
        w0k8 = wpk[0:64][:, 256:320]
        wdp = wpk[:][:, 320:640]
        wdn = wpk[:][:, 640:960]
        w1t = wpk[0:65][:, 1056:1088]
        w3ty = wpk[0:32][:, 1088:1728]
        w3tx = wpk[0:32][:, 1728:2368]
        w2k = wpk[0:32][:, 2368:2656]
        bc = pw.tile([64, 2], DT_F32, tag="bc")
        b3y = pw.tile([128, 5], DT_F32, tag="b3y")
        b3x = pw.tile([128, 5], DT_F32, tag="b3x")
        mask = pw.tile([32, 34], DT_F32, tag="mask")
        zb = pw.tile([128, 1], DT_F32, tag="zb")
        mb = pw.tile([128, 1], DT_F32, tag="mb")

        # ---- persistent tensors ----
        # X9: per-k shifted windows of x, column-deinterleaved (h planes) so
        # every deform read is contiguous along jc -> DVE 2x mode.
        # x9v[t][p=(kk,c), h, u, jc] = xslab[c, u+ky, 2*jc + h + kx]
        x9 = [pp.tile([128, 2 * 68 * 66], DT_BF, tag=f"x9_{t}", name=f"x9_{t}")
              for t in range(5)]
        x9v = [a[:].rearrange("p (h u c) -> p h u c", h=2, u=68) for a in x9]
        offs = {("y", t): pp.tile([128, 2048], DT_BF, tag=f"offy_{t}",
                                  name=f"offy_{t}") for t in range(5)}
        offs.update({("x", t): pp.tile([128, 2048], DT_BF, tag=f"offx_{t}",
                                       name=f"offx_{t}") for t in range(5)})

        c2_stack = ExitStack()
        pc2o = c2_stack.enter_context(tc.tile_pool(name="c2o", bufs=1))
        c2out = pc2o.tile([32, 32 * 64], DT_BF, tag="c2out")
        c1out = pc2o.tile([32, 34 * 66], DT_BF, tag="c1out")
        c1outv = c1out[:].rearrange("p (m v) -> p m v", v=66)
        conv_stack = ExitStack()
        pconv = conv_stack.enter_context(tc.tile_pool(name="conv", bufs=1))
        # double-shift replicated slabs: xsb2 = slab | slab col+1;
        # xsbB = slab col+2 | slab row+1  (for conv0 K=128 pairs, no X9 dep)
        xsb2 = pconv.tile([128, 9240], DT_BF, tag="xsb2")
        xsb2v = xsb2[:].rearrange("p (h u c) -> p h u c", h=2, u=70)
        xsbB = pconv.tile([128, 9240], DT_BF, tag="xsbB")
        xsbBv = xsbB[:].rearrange("p (h u c) -> p h u c", h=2, u=70)
        pooled = pconv.tile([65, 34 * 64], DT_BF, tag="pooled")
        pooledv = pooled[:].rearrange("p (m j) -> p m j", j=64)

        # ---- phase A: DMAs with strict queue roles ----
        # sync: weights + conv inputs; gpsimd: the 9 X9 copies (deform-only);
        # scalar: only the tiny bias vector, then pure compute.
        nc.scalar.dma_start(bc[:], bc_d[:])
        nc.sync.dma_start(wpk[:], wpack_d[:])
        nc.sync.dma_start(xsb2[0:64, :], xslab_d[0:64, 0:9240])
        nc.sync.dma_start(xsbB[0:64, :], xslab_d[0:64, 1:9241])
        nc.gpsimd.dma_start(xsb2[64:128, 0:4620], xslab_d[0:64, 4620:9240])
        nc.gpsimd.dma_start(xsb2[64:128, 4620:9240], xslab_d[0:64, 1:4621])
        nc.gpsimd.dma_start(xsbB[64:128, :], xslab_d[0:64, 66:9306])
        nc.gpsimd.memset(zb[:], 0.0)
        nc.gpsimd.memset(mb[:], -1.0)
        nc.sync.dma_start(c1out[:], fill_d[0:32, 8976:8976 + 2244])  # ring 0s
        nc.sync.dma_start(pooled[64:65, :], fill_d[0:1, 0:2176])  # conv1 bias
        for t_, d_ in ((b3y, b3y_d), (b3x, b3x_d), (mask, mask_d)):
            nc.sync.dma_start(t_[:], d_[:])
        # X9 streams on gpsimd, t-ascending (deform consumes in t order);
        # 12 dma_starts: planes merge (+4620 src stride) except kx==1.
        for t in range(5):
            for kk in range(2):
                k = 2 * t + kk
                if k >= KK:
                    continue
                ky, kx = k // 3, k % 3
                if kx != 1:
                    base = ky * 66 + (1 if kx == 2 else 0)
                    srcv = xslab_d[0:64, base:base + 9240].rearrange(
                        "p (a b) -> p a b", a=2, b=4620)[:, :, 0:4488]
                    nc.gpsimd.dma_start(
                        x9[t][64 * kk:64 * kk + 64, :].rearrange(
                            "p (h c) -> p h c", h=2), srcv)
                else:
                    for h in range(2):
                        sh = h + kx
                        soff = (sh & 1) * 4620 + ky * 66 + sh // 2
                        nc.gpsimd.dma_start(
                            x9[t][64 * kk:64 * kk + 64,
                                  4488 * h:4488 * h + 4488],
                            xslab_d[0:64, soff:soff + 4488])
        # chunk4 rows 64..127: anything finite (wd rows are 0); use fill
        nc.gpsimd.dma_start(x9[4][64:128, :], fill_d[0:64, 0:8976])

        # ---- phase C: conv0 (K=128 pairs from xsb2/xsbB) + maxpool ----
        with tc.tile_pool(name="c0", bufs=3) as pc0, \
             tc.tile_pool(name="ps_c0", bufs=3, space=bass.MemorySpace.PSUM) as ps0p:
            for g in range(9):  # 8 conv0 rows -> 4 pooled rows (last: 4->2)
                # psum cols = (w-parity:2, u:nr, wc:64); wpar 0 = odd out col
                u0, nr = 8 * g, min(8, 68 - 8 * g)
                s0 = pc0.tile([64, 1024], DT_BF, tag="s0")
                for wpar, (ph, jc0) in enumerate(((0, 1), (1, 0))):
                    ps0 = ps0p.tile([64, 512], DT_F32, tag="ps0")
                    for i in range(4):  # K=128 pairs (2i, 2i+1)
                        if i == 1:  # pair (2,3) lives in xsbB at base AP
                            rhs = xsbBv[:, ph, u0:u0 + nr, jc0:jc0 + 64]
                        else:
                            k = 2 * i
                            ky, sh = k // 3, ph + k % 3
                            rhs = xsb2v[:, sh & 1, u0 + ky:u0 + ky + nr,
                                        jc0 + sh // 2:jc0 + sh // 2 + 64]
                        nc.tensor.matmul(ps0[:, 0:64 * nr],
                                         w0p[:, 64 * i:64 * i + 64], rhs,
                                         start=(i == 0), stop=False)
                    sh = ph + 2  # k8 = (2,2): K=64 from the raw half
                    nc.tensor.matmul(
                        ps0[:, 0:64 * nr], w0k8,
                        xsb2v[0:64, sh & 1, u0 + 2:u0 + 2 + nr,
                              jc0 + sh // 2:jc0 + sh // 2 + 64],
                        start=False, stop=True)
                    nc.scalar.activation(
                        s0[:, 512 * wpar:512 * wpar + 64 * nr],
                        ps0[:, 0:64 * nr], ACTF.Identity,
                        bias=bc[:][:, 0:1], scale=1.0)
                p1 = pc0.tile([64, 512], DT_BF, tag="p1")
                p1v = p1[:].rearrange("p (u j) -> p u j", j=64)
                nc.vector.tensor_max(p1[:, 0:64 * nr], s0[:, 0:64 * nr],
                                     s0[:, 512:512 + 64 * nr])
                nc.vector.tensor_max(
                    pooledv[0:64, 4 * g:4 * g + nr // 2, :],
                    p1v[:, 0:nr:2, :], p1v[:, 1:nr:2, :])

        # ---- phase D: conv1 + row mask ----
        with tc.tile_pool(name="ps_c1", bufs=2, space=bass.MemorySpace.PSUM) as ps1p:
            mrows = [(0, 8), (8, 8), (16, 8), (24, 8), (32, 2)]
            for m0, mr in mrows:
                ps1 = ps1p.tile([32, 512], DT_F32, tag="ps1")
                nc.tensor.matmul(ps1[:, :mr * 64], w1t,
                                 pooledv[:, m0:m0 + mr, :],
                                 start=True, stop=True)
                nc.vector.tensor_mul(
                    c1outv[:, m0:m0 + mr, 1:65],
                    ps1[:, :mr * 64].rearrange("p (m j) -> p m j", j=64),
                    mask[:][:, m0:m0 + mr].unsqueeze(2).broadcast_to([32, mr, 64]))
        conv_stack.close()

        # ---- phases E..G: conv2/conv3 pipelined into the deformable conv ----
        with tc.tile_pool(name="hats", bufs=1) as phat, \
             tc.tile_pool(name="cwp", bufs=1) as pcw, \
             tc.tile_pool(name="zp", bufs=2) as pz, \
             tc.tile_pool(name="outp", bufs=2) as po:

            def emit_hats(u):
                s, t = divmod(u, 5)
                hats, hv, osl, ha = {}, {}, {}, {}
                for ax in ("y", "x"):
                    osl[ax] = offs[(ax, t)][:].rearrange(
                        "p (i j) -> p i j", j=64)[:, 16 * s:16 * s + 16, :]
                    hats[ax] = phat.tile([128, 3072], DT_BF, tag=f"h{ax}",
                                         name=f"h{ax}")
                    ha[ax] = phat.tile([128, 1024], DT_BF, tag="hsc",
                                       name="hsc", bufs=1)
                    hv[ax] = hats[ax][:].rearrange("p (r i j) -> p r i j",
                                                   r=3, j=64)
                # ordered so cw(ry=-1) unblocks after 5 acts
                nc.scalar.activation(hv["y"][:, 0], osl["y"], ACTF.Relu,
                                     bias=zb[:], scale=-1.0)
                for ax in ("x", "y"):
                    if ax == "x":
                        nc.scalar.activation(hv[ax][:, 0], osl[ax], ACTF.Relu,
                                             bias=zb[:], scale=-1.0)
                    nc.scalar.activation(hv[ax][:, 2], osl[ax], ACTF.Relu,
                                         bias=zb[:])
                    nc.scalar.activation(
                        ha[ax][:].rearrange("p (i j) -> p i j", j=64),
                        osl[ax], ACTF.Abs, bias=zb[:])
                    # negated wy0: |off| - 1
                    nc.scalar.activation(hats[ax][:][:, 1024:2048], ha[ax][:],
                                         ACTF.Identity, bias=mb[:], scale=1.0)
                return hats

            def emit_cw(hats):
                cw = {}
                hyv = hats["y"][:].rearrange("p (r q) -> p r q", r=3)
                for ry in (-1, 1, 0):
                    cwt = pcw.tile([128, 3072], DT_BF, tag=f"cw{ry}",
                                   name=f"cw{ry}")
                    nc.vector.tensor_mul(
                        cwt[:].rearrange("p (x q) -> p x q", x=3),
                        hyv[:, ry + 1:ry + 2, :].broadcast_to([128, 3, 1024]),
                        hats["x"][:].rearrange("p (x q) -> p x q", x=3))
                    cw[ry] = cwt
                return cw

            def emit_conv2(ps2p, nts):
                for nt in nts:
                    ps2 = ps2p.tile([32, 512], DT_F32, tag="ps2")
                    for k in range(KK):
                        ky, kx = k // 3, k % 3
                        nc.tensor.matmul(
                            ps2[:], w2k[:, 32 * k:32 * k + 32],
                            c1outv[0:32, 8 * nt + ky:8 * nt + ky + 8,
                                   kx:kx + 64],
                            start=(k == 0), stop=(k == KK - 1))
                    nc.scalar.activation(
                        c2out[:, 512 * nt:512 * nt + 512], ps2[:],
                        ACTF.Identity, bias=bc[0:32][:, 1:2], scale=1.0)

            def emit_conv3(ps3p, hf, ts):
                for t in ts:
                    for ax, wsb, bsb in (("y", w3ty, b3y), ("x", w3tx, b3x)):
                        ps3 = ps3p.tile([128, 1024], DT_F32, tag="ps3")
                        for m in range(2):
                            nc.tensor.matmul(
                                ps3[:, 512 * m:512 * m + 512],
                                wsb[:, 128 * t:128 * t + 128],
                                c2out[:, 1024 * hf + 512 * m:
                                      1024 * hf + 512 * m + 512],
                                start=True, stop=True)
                        if hf == 0:
                            nc.scalar.activation(
                                offs[(ax, t)][:, 1024 * hf:1024 * hf + 1024],
                                ps3[:], ACTF.Identity,
                                bias=bsb[:][:, t:t + 1], scale=1.0)
                        else:
                            nc.vector.scalar_tensor_tensor(
                                offs[(ax, t)][:, 1024 * hf:1024 * hf + 1024],
                                ps3[:], 0.0,
                                bsb[:][:, t:t + 1].broadcast_to([128, 1024]),
                                op0=ALU.add, op1=ALU.add)

            with tc.tile_pool(name="ps_c2", bufs=2,
                              space=bass.MemorySpace.PSUM) as ps2p, \
                 tc.tile_pool(name="ps_c3", bufs=3,
                              space=bass.MemorySpace.PSUM) as ps3p:
                emit_conv2(ps2p, [0, 1])
                emit_conv3(ps3p, 0, [0])   # slab-0 t0 offsets first
                hats_cur = emit_hats(0)
                cw_cur = emit_cw(hats_cur)
                emit_conv2(ps2p, [2, 3])
                emit_conv3(ps3p, 1, range(5))
                emit_conv3(ps3p, 0, [1, 2, 3, 4])
                hats_nxt = emit_hats(1)

            with tc.tile_pool(name="ps_d", bufs=1,
                              space=bass.MemorySpace.PSUM) as psdp:
                psd = None
                for u in range(10):
                    s, t = divmod(u, 5)
                    if t == 0:
                        psd = psdp.tile([64, 4096], DT_F32, tag="psd")
                    if u > 0:
                        cw_cur = emit_cw(hats_cur)
                    for ci, (ry, rx) in enumerate(CORNERS):
                        cwv = cw_cur[ry][:].rearrange(
                            "p (x I j) -> p x I j", x=3, I=16)[:, rx + 1]
                        # z layout [128, n(32)=(I,r), tj(2), j(64)]
                        z = pz.tile([128, 4096], DT_BF, tag="z")
                        zv = z[:].rearrange("p (I r tj j) -> p I r tj j",
                                            I=16, r=2, tj=2)
                        u0 = 32 * s + 2 + ry
                        for tj in range(2):
                            if rx != 0:
                                vh, jc0 = tj, (rx + 1) >> 1
                            else:
                                vh, jc0 = (tj + 1) & 1, (tj + 1) >> 1
                            nc.vector.tensor_mul(
                                zv[:, :, :, tj, :],
                                cwv.unsqueeze(2).broadcast_to([128, 16, 2, 64]),
                                x9v[t][:, vh, u0:u0 + 32,
                                       jc0:jc0 + 64].rearrange(
                                    "p (I r) c -> p I r c", I=16))
                        if ci == 2 and u < 9:
                            # lookahead: next unit's hats overlap these z muls
                            if u > 0:
                                hats_nxt = emit_hats(u + 1)
                            hats_cur = hats_nxt
                        wsel = wdp if (ry == 0) == (rx == 0) else wdn
                        first = (t == 0) and ci == 0
                        last = (t == 4) and ci == len(CORNERS) - 1

                        def emit_out(oh):
                            osb = po.tile([64, 2048], DT_BF, tag="osb")
                            nc.scalar.copy(
                                osb[:].rearrange(
                                    "p (I j tj) -> p I tj j", I=16, tj=2),
                                psd[:, 2048 * oh:2048 * oh + 2048].rearrange(
                                    "p (I tj j) -> p I tj j", I=16, tj=2))
                            nc.sync.dma_start(
                                out_d[:, 4096 * s + 2048 * oh:
                                      4096 * s + 2048 * oh + 2048],
                                osb[:])

                        for q in range(8):
                            nc.tensor.matmul(
                                psd[:, 512 * q:512 * q + 512],
                                wsel[:, 64 * t:64 * t + 64],
                                z[:, 512 * q:512 * q + 512],
                                start=first, stop=last)
                            if last and q == 3:
                                emit_out(0)  # drain oh0 while q4-7 run
                        if last:
                            emit_out(1)
        c2_stack.close()


# ----------------------------------------------------------------------------
# Entry point.
# ----------------------------------------------------------------------------

def kernel(**inputs):
    if "nc" not in _CACHE:
        _CACHE["nc"] = build_nc()
    nc = _CACHE["nc"]
    in_maps = host_prepro(inputs)
    res = run_bass_kernel_spmd(nc, in_maps, list(range(NCORES))).results
    out = np.zeros((4, 64, 128, 128), F32)
    for core in range(NCORES):
        b, h = core // 2, core % 2
        out[b, :, 64 * h:64 * h + 64, :] = np.asarray(
            res[core]["out"], F32).reshape(64, 64, 128)
    return out


# revision 23
# speedup vs baseline: 1.0015x; 1.0015x over previous
"""Trainium2 Bass kernel for nn_EDDeform (deformable-conv CNN).

Sharding: 8 cores = (batch b in 0..3) x (output-row half h in 0..1).
Each core computes output rows [64h, 64h+64) of sample b from a padded
x slab, running the full offset chain (conv0 -> maxpool -> conv1 ->
conv2 -> conv3) and the deformable conv.

Deformable gather: offsets satisfy |off| < 1 for these inputs, so the
bilinear sample is a 3x3 "hat" stencil with separable weights
  wy[-1] = relu(-oy), wy[+1] = relu(oy), wy[0] = 1 - |oy|
(all statically addressed), folded into 9 PSUM-accumulated matmuls of
K=576 (5 chunks of 128 = k-major (k,c) pairs) against per-k shifted
copies of x (X9 windows). wy[0] is computed negated (one fused DVE op);
the sign is absorbed by negated deform weights for the 4 affected
corners.

Schedule: dma_start blocks the issuing engine's queue, so queues have
strict roles -- Scalar runs only compute (psum drains, offsets, hats),
Sync carries the small slab/weight DMAs, GpSimd streams the 9 shifted
X9 copies (deform-only).  conv0 runs from two double-shift-replicated
slab tiles (K=128 pairs, no X9 dependency), conv2 reads c1out windows
directly (no im2col gather), and conv3-t0's offsets + the first hat
block are emitted mid-conv2 so DVE deform work starts early.  Hats for
unit u+1 are produced during unit u's z muls; slab PSUM drains overlap
the next slab's first hats.
"""
import sys
import numpy as np

if "/opt/trn_rl_repo" not in sys.path:
    sys.path.insert(0, "/opt/trn_rl_repo")

import ml_dtypes
import concourse.bass as bass
import concourse.bacc as bacc
import concourse.tile as tile
import concourse.mybir as mybir
from concourse.bass_utils import run_bass_kernel_spmd

BF16 = ml_dtypes.bfloat16
F32 = np.float32
DT_BF = mybir.dt.bfloat16
DT_F32 = mybir.dt.float32
ALU = mybir.AluOpType
ACTF = mybir.ActivationFunctionType

KK = 9
NCORES = 8

_CACHE = {}


# ----------------------------------------------------------------------------
# Host-side preprocessing (sharding + weight layout), numpy only.
# ----------------------------------------------------------------------------

def _ck(idx):
    return idx // 64, idx % 64  # k, c (k-major)


def host_prepro(inputs):
    x = np.asarray(inputs["x"], F32)          # [4, 64, 130, 130]
    B, C, H, W = x.shape

    xslabs = []
    for core in range(NCORES):
        b, h = core // 2, core % 2
        slab = np.zeros((C, 70, 132), F32)
        r0 = 64 * h - 2
        lo = max(0, -r0)
        hi = min(70, H - r0)
        slab[:, lo:hi, 1:131] = x[b, :, r0 + lo:r0 + hi, :]
        # deinterleave columns: [C, 2, 70, 66], plane hh holds cols 2*jc+hh
        slab = slab.reshape(C, 70, 66, 2).transpose(0, 3, 1, 2).reshape(C, 9240)
        slabp = np.zeros((C, 9376), F32)
        slabp[:, :9240] = slab
        xslabs.append(slabp.astype(BF16))

    w0, b0 = np.asarray(inputs["w0"], F32), np.asarray(inputs["b0"], F32)
    wd = np.asarray(inputs["wd"], F32)
    wdt9 = np.zeros((128, 5 * 64), F32)
    for t in range(5):
        for p in range(128):
            idx = 128 * t + p
            if idx < 576:
                k, c = _ck(idx)
                wdt9[p, 64 * t:64 * t + 64] = wd[:, c, k // 3, k % 3]

    w1, b1 = np.asarray(inputs["w1"], F32), np.asarray(inputs["b1"], F32)
    w1t = np.zeros((65, 32), F32)
    w1t[:64] = w1[:, :, 0, 0].T
    w1t[64] = b1

    w2, b2 = np.asarray(inputs["w2"], F32), np.asarray(inputs["b2"], F32)
    w3, b3 = np.asarray(inputs["w3"], F32), np.asarray(inputs["b3"], F32)
    w3ty = np.zeros((32, 5 * 128), F32)
    w3tx = np.zeros((32, 5 * 128), F32)
    b3y = np.zeros((128, 5), F32)
    b3x = np.zeros((128, 5), F32)
    for t in range(5):
        for p in range(128):
            idx = 128 * t + p
            if idx < 576:
                k, c = _ck(idx)
                chy = (c * KK + k) * 2
                w3ty[:, 128 * t + p] = w3[chy, :, 0, 0]
                w3tx[:, 128 * t + p] = w3[chy + 1, :, 0, 0]
                b3y[p, t] = b3[chy]
                b3x[p, t] = b3[chy + 1]

    masks = []
    for core in range(NCORES):
        h = core % 2
        m = np.ones((32, 34), F32)
        m[:, 0 if h == 0 else 33] = 0.0
        masks.append(m)

    fill = np.zeros((64, 11232), F32)
    fill[:, :8976] = 1.0

    # packed bf16 weights: w0pairs|w0k8|wdp|wdn|w1t|w3ty|w3tx|w2k
    wpack = np.zeros((128, 2656), F32)
    for i in range(4):  # conv0 K=128 pair lhsT: rows (c | c+64) = k=2i | 2i+1
        k0, k1 = 2 * i, 2 * i + 1
        wpack[0:64, 64 * i:64 * i + 64] = w0[:, :, k0 // 3, k0 % 3].T
        wpack[64:128, 64 * i:64 * i + 64] = w0[:, :, k1 // 3, k1 % 3].T
    wpack[0:64, 256:320] = w0[:, :, 2, 2].T  # k8
    wpack[:, 320:640] = wdt9
    wpack[:, 640:960] = -wdt9
    wpack[0:65, 1056:1088] = w1t
    wpack[0:32, 1088:1728] = w3ty
    wpack[0:32, 1728:2368] = w3tx
    for k in range(KK):  # conv2 per-k lhsT [c2, cout2]
        wpack[0:32, 2368 + 32 * k:2368 + 32 * k + 32] = w2[:, :, k // 3, k % 3].T

    bc = np.zeros((64, 2), F32)
    bc[:, 0] = b0
    bc[0:32, 1] = b2

    const = dict(
        wpack=wpack.astype(BF16), b3y=b3y, b3x=b3x, bc=bc,
        fill=fill.astype(BF16),
    )
    in_maps = []
    for core in range(NCORES):
        m = dict(const)
        m["xslab"] = xslabs[core]
        m["maskrow"] = masks[core]
        in_maps.append(m)
    return in_maps


# ----------------------------------------------------------------------------
# Bass kernel builder.
# ----------------------------------------------------------------------------

# corners grouped by ry so cw(ry=-1) (ready after 5 hat acts) goes first
CORNERS = [(-1, -1), (-1, 1), (1, -1), (1, 1), (-1, 0),
           (1, 0), (0, -1), (0, 1), (0, 0)]


def build_nc():
    nc = bacc.Bacc(None)

    xslab_d = nc.declare_dram_parameter("xslab", [64, 9376], DT_BF, isOutput=False)
    wpack_d = nc.declare_dram_parameter("wpack", [128, 2656], DT_BF, isOutput=False)
    bc_d = nc.declare_dram_parameter("bc", [64, 2], DT_F32, isOutput=False)
    b3y_d = nc.declare_dram_parameter("b3y", [128, 5], DT_F32, isOutput=False)
    b3x_d = nc.declare_dram_parameter("b3x", [128, 5], DT_F32, isOutput=False)
    mask_d = nc.declare_dram_parameter("maskrow", [32, 34], DT_F32, isOutput=False)
    fill_d = nc.declare_dram_parameter("fill", [64, 11232], DT_BF, isOutput=False)
    out_d = nc.declare_dram_parameter("out", [64, 64 * 128], DT_BF, isOutput=True)

    with tile.TileContext(nc) as tc:
        _body(nc, tc, xslab_d, wpack_d, bc_d, b3y_d, b3x_d, mask_d, fill_d,
              out_d)
    nc.compile()
    return nc


def _body(nc, tc, xslab_d, wpack_d, bc_d, b3y_d, b3x_d, mask_d, fill_d, out_d):
    from contextlib import ExitStack

    with ExitStack() as top:
        pw = top.enter_context(tc.tile_pool(name="weights", bufs=1))
        pp = top.enter_context(tc.tile_pool(name="persist", bufs=1))

        # ---- weights (one packed DMA) ----
        wpk = pw.tile([128, 2656], DT_BF, tag="wpack")
        w0p = wpk[:][:, 0:256]
        w0k8 = wpk[0:64][:, 256:320]
        wdp = wpk[:][:, 320:640]
        wdn = wpk[:][:, 640:960]
        w1t = wpk[0:65][:, 1056:1088]
        w3ty = wpk[0:32][:, 1088:1728]
        w3tx = wpk[0:32][:, 1728:2368]
        w2k = wpk[0:32][:, 2368:2656]
        bc = pw.tile([64, 2], DT_F32, tag="bc")
        b3y = pw.tile([128, 5], DT_F32, tag="b3y")
        b3x = pw.tile([128, 5], DT_F32, tag="b3x")
        mask = pw.tile([32, 34], DT_F32, tag="mask")
        zb = pw.tile([128, 1], DT_F32, tag="zb")
        mb = pw.tile([128, 1], DT_F32, tag="mb")

        # ---- persistent tensors ----
        # X9: per-k shifted windows of x, column-deinterleaved (h planes) so
        # every deform read is contiguous along jc -> DVE 2x mode.
        # x9v[t][p=(kk,c), h, u, jc] = xslab[c, u+ky, 2*jc + h + kx]
        x9 = [pp.tile([128, 2 * 68 * 66], DT_BF, tag=f"x9_{t}", name=f"x9_{t}")
              for t in range(5)]
        x9v = [a[:].rearrange("p (h u c) -> p h u c", h=2, u=68) for a in x9]
        offs = {("y", t): pp.tile([128, 2048], DT_BF, tag=f"offy_{t}",
                                  name=f"offy_{t}") for t in range(5)}
        offs.update({("x", t): pp.tile([128, 2048], DT_BF, tag=f"offx_{t}",
                                       name=f"offx_{t}") for t in range(5)})

        c2_stack = ExitStack()
        pc2o = c2_stack.enter_context(tc.tile_pool(name="c2o", bufs=1))
        c2out = pc2o.tile([32, 32 * 64], DT_BF, tag="c2out")
        c1out = pc2o.tile([32, 34 * 66], DT_BF, tag="c1out")
        c1outv = c1out[:].rearrange("p (m v) -> p m v", v=66)
        conv_stack = ExitStack()
        pconv = conv_stack.enter_context(tc.tile_pool(name="conv", bufs=1))
        # double-shift replicated slabs: xsb2 = slab | slab col+1;
        # xsbB = slab col+2 | slab row+1  (for conv0 K=128 pairs, no X9 dep)
        xsb2 = pconv.tile([128, 9240], DT_BF, tag="xsb2")
        xsb2v = xsb2[:].rearrange("p (h u c) -> p h u c", h=2, u=70)
        xsbB = pconv.tile([128, 9240], DT_BF, tag="xsbB")
        xsbBv = xsbB[:].rearrange("p (h u c) -> p h u c", h=2, u=70)
        pooled = pconv.tile([65, 34 * 64], DT_BF, tag="pooled")
        pooledv = pooled[:].rearrange("p (m j) -> p m j", j=64)

        # ---- phase A: DMAs with strict queue roles ----
        # sync: weights + conv inputs; gpsimd: the 9 X9 copies (deform-only);
        # scalar: only the tiny bias vector, then pure compute.
        nc.scalar.dma_start(bc[:], bc_d[:])
        nc.sync.dma_start(wpk[:], wpack_d[:])
        nc.sync.dma_start(xsb2[0:64, :], xslab_d[0:64, 0:9240])
        nc.sync.dma_start(xsbB[0:64, :], xslab_d[0:64, 1:9241])
        nc.gpsimd.dma_start(xsb2[64:128, 0:4620], xslab_d[0:64, 4620:9240])
        nc.gpsimd.dma_start(xsb2[64:128, 4620:9240], xslab_d[0:64, 1:4621])
        nc.gpsimd.dma_start(xsbB[64:128, :], xslab_d[0:64, 66:9306])
        nc.gpsimd.memset(zb[:], 0.0)
        nc.gpsimd.memset(mb[:], -1.0)
        nc.sync.dma_start(c1out[:], fill_d[0:32, 8976:8976 + 2244])  # ring 0s
        nc.sync.dma_start(pooled[64:65, :], fill_d[0:1, 0:2176])  # conv1 bias
        for t_, d_ in ((b3y, b3y_d), (b3x, b3x_d), (mask, mask_d)):
            nc.sync.dma_start(t_[:], d_[:])
        # X9 streams on gpsimd, t-ascending (deform consumes in t order);
        # 12 dma_starts: planes merge (+4620 src stride) except kx==1.
        for t in range(5):
            for kk in range(2):
                k = 2 * t + kk
                if k >= KK:
                    continue
                ky, kx = k // 3, k % 3
                if kx != 1:
                    base = ky * 66 + (1 if kx == 2 else 0)
                    srcv = xslab_d[0:64, base:base + 9240].rearrange(
                        "p (a b) -> p a b", a=2, b=4620)[:, :, 0:4488]
                    nc.gpsimd.dma_start(
                        x9[t][64 * kk:64 * kk + 64, :].rearrange(
                            "p (h c) -> p h c", h=2), srcv)
                else:
                    for h in range(2):
                        sh = h + kx
                        soff = (sh & 1) * 4620 + ky * 66 + sh // 2
                        nc.gpsimd.dma_start(
                            x9[t][64 * kk:64 * kk + 64,
                                  4488 * h:4488 * h + 4488],
                            xslab_d[0:64, soff:soff + 4488])
        # chunk4 rows 64..127: anything finite (wd rows are 0); use fill
        nc.gpsimd.dma_start(x9[4][64:128, :], fill_d[0:64, 0:8976])

        # ---- phase C: conv0 (K=128 pairs from xsb2/xsbB) + maxpool ----
        with tc.tile_pool(name="c0", bufs=3) as pc0, \
             tc.tile_pool(name="ps_c0", bufs=3, space=bass.MemorySpace.PSUM) as ps0p:
            for g in range(9):  # 8 conv0 rows -> 4 pooled rows (last: 4->2)
                # psum cols = (w-parity:2, u:nr, wc:64); wpar 0 = odd out col
                u0, nr = 8 * g, min(8, 68 - 8 * g)
                s0 = pc0.tile([64, 1024], DT_BF, tag="s0")
                for wpar, (ph, jc0) in enumerate(((0, 1), (1, 0))):
                    ps0 = ps0p.tile([64, 512], DT_F32, tag="ps0")
                    for i in range(4):  # K=128 pairs (2i, 2i+1)
                        if i == 1:  # pair (2,3) lives in xsbB at base AP
                            rhs = xsbBv[:, ph, u0:u0 + nr, jc0:jc0 + 64]
                        else:
                            k = 2 * i
                            ky, sh = k // 3, ph + k % 3
                            rhs = xsb2v[:, sh & 1, u0 + ky:u0 + ky + nr,
                                        jc0 + sh // 2:jc0 + sh // 2 + 64]
                        nc.tensor.matmul(ps0[:, 0:64 * nr],
                                         w0p[:, 64 * i:64 * i + 64], rhs,
                                         start=(i == 0), stop=False)
                    sh = ph + 2  # k8 = (2,2): K=64 from the raw half
                    nc.tensor.matmul(
                        ps0[:, 0:64 * nr], w0k8,
                        xsb2v[0:64, sh & 1, u0 + 2:u0 + 2 + nr,
                              jc0 + sh // 2:jc0 + sh // 2 + 64],
                        start=False, stop=True)
                    nc.scalar.activation(
                        s0[:, 512 * wpar:512 * wpar + 64 * nr],
                        ps0[:, 0:64 * nr], ACTF.Identity,
                        bias=bc[:][:, 0:1], scale=1.0)
                p1 = pc0.tile([64, 512], DT_BF, tag="p1")
                p1v = p1[:].rearrange("p (u j) -> p u j", j=64)
                nc.vector.tensor_max(p1[:, 0:64 * nr], s0[:, 0:64 * nr],
                                     s0[:, 512:512 + 64 * nr])
                nc.vector.tensor_max(
                    pooledv[0:64, 4 * g:4 * g + nr // 2, :],
                    p1v[:, 0:nr:2, :], p1v[:, 1:nr:2, :])

        # ---- phase D: conv1 + row mask ----
        with tc.tile_pool(name="ps_c1", bufs=2, space=bass.MemorySpace.PSUM) as ps1p:
            mrows = [(0, 8), (8, 8), (16, 8), (24, 8), (32, 2)]
            for m0, mr in mrows:
                ps1 = ps1p.tile([32, 512], DT_F32, tag="ps1")
                nc.tensor.matmul(ps1[:, :mr * 64], w1t,
                                 pooledv[:, m0:m0 + mr, :],
                                 start=True, stop=True)
                nc.vector.tensor_mul(
                    c1outv[:, m0:m0 + mr, 1:65],
                    ps1[:, :mr * 64].rearrange("p (m j) -> p m j", j=64),
                    mask[:][:, m0:m0 + mr].unsqueeze(2).broadcast_to([32, mr, 64]))
        conv_stack.close()

        # ---- phases E..G: conv2/conv3 pipelined into the deformable conv ----
        with tc.tile_pool(name="hats", bufs=1) as phat, \
             tc.tile_pool(name="cwp", bufs=1) as pcw, \
             tc.tile_pool(name="zp", bufs=2) as pz, \
             tc.tile_pool(name="outp", bufs=2) as po:

            def emit_hats(u):
                s, t = divmod(u, 5)
                hats, hv, osl, ha = {}, {}, {}, {}
                for ax in ("y", "x"):
                    osl[ax] = offs[(ax, t)][:].rearrange(
                        "p (i j) -> p i j", j=64)[:, 16 * s:16 * s + 16, :]
                    hats[ax] = phat.tile([128, 3072], DT_BF, tag=f"h{ax}",
                                         name=f"h{ax}")
                    ha[ax] = phat.tile([128, 1024], DT_BF, tag="hsc",
                                       name="hsc", bufs=1)
                    hv[ax] = hats[ax][:].rearrange("p (r i j) -> p r i j",
                                                   r=3, j=64)
                # ordered so cw(ry=-1) unblocks after 5 acts
                nc.scalar.activation(hv["y"][:, 0], osl["y"], ACTF.Relu,
                                     bias=zb[:], scale=-1.0)
                for ax in ("x", "y"):
                    if ax == "x":
                        nc.scalar.activation(hv[ax][:, 0], osl[ax], ACTF.Relu,
                                             bias=zb[:], scale=-1.0)
                    nc.scalar.activation(hv[ax][:, 2], osl[ax], ACTF.Relu,
                                         bias=zb[:])
                    nc.scalar.activation(
                        ha[ax][:].rearrange("p (i j) -> p i j", j=64),
                        osl[ax], ACTF.Abs, bias=zb[:])
                    # negated wy0: |off| - 1
                    nc.scalar.activation(hats[ax][:][:, 1024:2048], ha[ax][:],
                                         ACTF.Identity, bias=mb[:], scale=1.0)
                return hats

            def emit_cw(hats):
                cw = {}
                hyv = hats["y"][:].rearrange("p (r q) -> p r q", r=3)
                for ry in (-1, 1, 0):
                    cwt = pcw.tile([128, 3072], DT_BF, tag=f"cw{ry}",
                                   name=f"cw{ry}")
                    nc.vector.tensor_mul(
                        cwt[:].rearrange("p (x q) -> p x q", x=3),
                        hyv[:, ry + 1:ry + 2, :].broadcast_to([128, 3, 1024]),
                        hats["x"][:].rearrange("p (x q) -> p x q", x=3))
                    cw[ry] = cwt
                return cw

            def emit_conv2(ps2p, nts):
                for nt in nts:
                    ps2 = ps2p.tile([32, 512], DT_F32, tag="ps2")
                    for k in range(KK):
                        ky, kx = k // 3, k % 3
                        nc.tensor.matmul(
                            ps2[:], w2k[:, 32 * k:32 * k + 32],
                            c1outv[0:32, 8 * nt + ky:8 * nt + ky + 8,
                                   kx:kx + 64],
                            start=(k == 0), stop=(k == KK - 1))
                    nc.scalar.activation(
                        c2out[:, 512 * nt:512 * nt + 512], ps2[:],
                        ACTF.Identity, bias=bc[0:32][:, 1:2], scale=1.0)

            def emit_conv3(ps3p, hf, ts):
                for t in ts:
                    for ax, wsb, bsb in (("y", w3ty, b3y), ("x", w3tx, b3x)):
                        ps3 = ps3p.tile([128, 1024], DT_F32, tag="ps3")
                        for m in range(2):
                            nc.tensor.matmul(
                                ps3[:, 512 * m:512 * m + 512],
                                wsb[:, 128 * t:128 * t + 128],
                                c2out[:, 1024 * hf + 512 * m:
                                      1024 * hf + 512 * m + 512],
                                start=True, stop=True)
                        nc.scalar.activation(
                            offs[(ax, t)][:, 1024 * hf:1024 * hf + 1024],
                            ps3[:], ACTF.Identity,
                            bias=bsb[:][:, t:t + 1], scale=1.0)

            with tc.tile_pool(name="ps_c2", bufs=2,
                              space=bass.MemorySpace.PSUM) as ps2p, \
                 tc.tile_pool(name="ps_c3", bufs=3,
                              space=bass.MemorySpace.PSUM) as ps3p:
                emit_conv2(ps2p, [0, 1])
                emit_conv3(ps3p, 0, [0])   # slab-0 t0 offsets first
                hats_cur = emit_hats(0)
                cw_cur = emit_cw(hats_cur)
                emit_conv2(ps2p, [2, 3])
                emit_conv3(ps3p, 1, range(5))
                emit_conv3(ps3p, 0, [1, 2, 3, 4])
                hats_nxt = emit_hats(1)

            with tc.tile_pool(name="ps_d", bufs=1,
                              space=bass.MemorySpace.PSUM) as psdp:
                psd = None
                for u in range(10):
                    s, t = divmod(u, 5)
                    if t == 0:
                        psd = psdp.tile([64, 4096], DT_F32, tag="psd")
                    if u > 0:
                        cw_cur = emit_cw(hats_cur)
                    for ci, (ry, rx) in enumerate(CORNERS):
                        cwv = cw_cur[ry][:].rearrange(
                            "p (x I j) -> p x I j", x=3, I=16)[:, rx + 1]
                        # z layout [128, n(32)=(I,r), tj(2), j(64)]
                        z = pz.tile([128, 4096], DT_BF, tag="z")
                        zv = z[:].rearrange("p (I r tj j) -> p I r tj j",
                                            I=16, r=2, tj=2)
                        u0 = 32 * s + 2 + ry
                        for tj in range(2):
                            if rx != 0:
                                vh, jc0 = tj, (rx + 1) >> 1
                            else:
                                vh, jc0 = (tj + 1) & 1, (tj + 1) >> 1
                            nc.vector.tensor_mul(
                                zv[:, :, :, tj, :],
                                cwv.unsqueeze(2).broadcast_to([128, 16, 2, 64]),
                                x9v[t][:, vh, u0:u0 + 32,
                                       jc0:jc0 + 64].rearrange(
                                    "p (I r) c -> p I r c", I=16))
                        if ci == 2 and u < 9:
                            # lookahead: next unit's hats overlap these z muls
                            if u > 0:
                                hats_nxt = emit_hats(u + 1)
                            hats_cur = hats_nxt
                        wsel = wdp if (ry == 0) == (rx == 0) else wdn
                        first = (t == 0) and ci == 0
                        last = (t == 4) and ci == len(CORNERS) - 1

                        def emit_out(oh):
                            osb = po.tile([64, 2048], DT_BF, tag="osb")
                            nc.scalar.copy(
                                osb[:].rearrange(
                                    "p (I j tj) -> p I tj j", I=16, tj=2),
                                psd[:, 2048 * oh:2048 * oh + 2048].rearrange(
                                    "p (I tj j) -> p I tj j", I=16, tj=2))
                            nc.sync.dma_start(
                                out_d[:, 4096 * s + 2048 * oh:
                                      4096 * s + 2048 * oh + 2048],
                                osb[:])

                        for q in range(8):
                            nc.tensor.matmul(
                                psd[:, 512 * q:512 * q + 512],
                                wsel[:, 64 * t:64 * t + 64],
                                z[:, 512 * q:512 * q + 512],
                                start=first, stop=last)
                            if last and q == 3:
                                emit_out(0)  # drain oh0 while q4-7 run
                        if last:
                            emit_out(1)
        c2_stack.close()


# ----------------------------------------------------------------------------
# Entry point.
# ----------------------------------------------------------------------------

def kernel(**inputs):
    if "nc" not in _CACHE:
        _CACHE["nc"] = build_nc()
    nc = _CACHE["nc"]
    in_maps = host_prepro(inputs)
    res = run_bass_kernel_spmd(nc, in_maps, list(range(NCORES))).results
    out = np.zeros((4, 64, 128, 128), F32)
    for core in range(NCORES):
        b, h = core // 2, core % 2
        out[b, :, 64 * h:64 * h + 64, :] = np.asarray(
            res[core]["out"], F32).reshape(64, 64, 128)
    return out
